# revision 8
# baseline (speedup 1.0000x reference)
"""Trainium2 Bass kernel for nn_AttentionBlock (B=2, C=1024, T=2048, H=16, GN32).

Sharding: B*H = 32 heads across 8 cores -> 4 heads/core (core i: batch i//4,
heads 4*(i%4) .. 4*(i%4)+3).  Each core:
  - computes GroupNorm(x[b]) fully (stats replicated per batch group),
  - computes its 768 qkv rows (weights pre-transposed+permuted on host),
  - attention per head in transposed-score layout: scoresT[s,t] = k^T q,
    exp on ScalarE (scale=1/8 folded in), softmax denominator obtained by
    appending a ones-column to v^T in the V-matmul, mask folded into v,
    normalization applied post-V-matmul (deferred divide),
  - partial projection proj_w[:, slice] @ a_slice  -> [1024, 2048].
Host sums the 4 partials per batch + residual + proj bias.
"""

import math

import numpy as np

import concourse.bass as bass
import concourse.tile as tile
from concourse import bacc, mybir
from concourse.bass_utils import run_bass_kernel_spmd

# ---------------------------------------------------------------- constants
B, C, T, H = 2, 1024, 2048, 16
GROUPS = 32
EPS = 1e-5
CH = C // H              # 64 head dim
P = 128
NCORES = 8
GPC = NCORES // B        # 4 cores per batch sample
HPC = H // GPC           # 4 heads per core
CT = C // P              # 8 channel tiles
QKV_ROWS = HPC * 3 * CH  # 768
QT = QKV_ROWS // P       # 6 qkv row tiles
ASL = HPC * CH           # 256 local a-channels
TC = 512                 # matmul N chunk
NTC = T // TC            # 4
TCB = 1024               # exp / score chunk (2 psum banks)
NTCB = T // TCB          # 2
ST = T // P              # 16 s-tiles
NG_ELEMS = (C // GROUPS) * T  # elements per group norm group

F32 = mybir.dt.float32
F32R = mybir.dt.float32r
AF = mybir.ActivationFunctionType
OP = mybir.AluOpType
AX = mybir.AxisListType

USE_FP32R = True         # fast fp32 matmul mode (toggle for accuracy tests)


def _r(ap):
    return ap.bitcast(F32R) if USE_FP32R else ap


WDT = F32R if USE_FP32R else F32  # dtype for matmul-feeding weight tensors


# ---------------------------------------------------------------- program
def build_program():
    nc = bacc.Bacc("TRN2", target_bir_lowering=False, debug=False,
                   num_devices=NCORES)

    x_d = nc.dram_tensor("x", [C, T], F32, kind="ExternalInput").ap()
    wq_d = nc.dram_tensor("wqkvT", [C, QKV_ROWS], WDT, kind="ExternalInput").ap()
    bq_d = nc.dram_tensor("bqkvT", [P, QT], F32, kind="ExternalInput").ap()
    pj_d = nc.dram_tensor("projT", [ASL, C], WDT, kind="ExternalInput").ap()
    mk_d = nc.dram_tensor("mask", [1, T], F32, kind="ExternalInput").ap()
    gw_d = nc.dram_tensor("gnw", [P, CT], F32, kind="ExternalInput").ap()
    gb_d = nc.dram_tensor("gnb", [P, CT], F32, kind="ExternalInput").ap()
    id_d = nc.dram_tensor("ident", [P, P], WDT, kind="ExternalInput").ap()
    i32_d = nc.dram_tensor("ind32", [P, 4], F32, kind="ExternalInput").ap()
    i2_d = nc.dram_tensor("i2bc", [4, P], F32, kind="ExternalInput").ap()
    out_d = nc.dram_tensor("out", [C, T], F32, kind="ExternalOutput").ap()

    with tile.TileContext(nc) as tc:
        from contextlib import ExitStack
        es = ExitStack()
        with es:
            persist = es.enter_context(tc.tile_pool(name="persist", bufs=1))
            pool_x = tc.alloc_tile_pool(name="xpool", bufs=1)
            pool_w = tc.alloc_tile_pool(name="wpool", bufs=1)
            pool_junk = tc.alloc_tile_pool(name="junk", bufs=1)
            psA = tc.alloc_tile_pool(name="psA", bufs=1, space="PSUM")

            # ---------------- loads
            xt = [pool_x.tile([P, T], F32, name=f"xt{j}", tag=f"xt{j}")
                  for j in range(CT)]
            for j in range(CT):
                nc.sync.dma_start(_r(xt[j][:]), _r(x_d[j * P:(j + 1) * P, :]))

            wq = [pool_w.tile([P, QKV_ROWS], WDT, name=f"wq{j}", tag=f"wq{j}")
                  for j in range(CT)]
            for j in range(CT):
                nc.sync.dma_start(wq[j][:], wq_d[j * P:(j + 1) * P, :])

            pj = [persist.tile([P, C], WDT, name=f"pj{k}", tag=f"pj{k}")
                  for k in range(2)]
            for k in range(2):
                nc.sync.dma_start(pj[k][:], pj_d[k * P:(k + 1) * P, :])

            bq_t = persist.tile([P, QT], F32, name="bq_t")
            nc.sync.dma_start(bq_t[:], bq_d[:])
            gnw_t = persist.tile([P, CT], F32, name="gnw_t")
            nc.sync.dma_start(gnw_t[:], gw_d[:])
            gnb_t = persist.tile([P, CT], F32, name="gnb_t")
            nc.sync.dma_start(gnb_t[:], gb_d[:])
            ident_t = persist.tile([P, P], WDT, name="ident_t")
            nc.sync.dma_start(ident_t[:], id_d[:])
            ind32_t = persist.tile([P, 4], F32, name="ind32_t")
            nc.sync.dma_start(ind32_t[:], i32_d[:])
            i2bc_t = persist.tile([4, P], F32, name="i2bc_t")
            nc.sync.dma_start(i2bc_t[:], i2_d[:])
            ones_c = persist.tile([P, 1], F32, name="ones_c")
            nc.vector.memset(ones_c[:], 1.0)
            mrow = persist.tile([1, T], F32, name="mrow")
            nc.sync.dma_start(mrow[:], mk_d[:])
            maskb = persist.tile([P, T], F32, name="maskb")
            nc.gpsimd.partition_broadcast(maskb[:], mrow[:])

            # ---------------- phase A: group norm stats
            stats = persist.tile([P, 2 * CT], F32, name="stats")
            for j in range(CT):
                nc.vector.tensor_reduce(stats[:, j:j + 1], xt[j][:],
                                        axis=AX.X, op=OP.add)
                junk = pool_junk.tile([P, T], F32, name="junk", tag="junk")
                nc.scalar.activation(junk[:], xt[j][:], AF.Square,
                                     accum_out=stats[:, CT + j:CT + j + 1])

            gstat = psA.tile([4, 2 * CT], F32, name="gstat", tag="gstat")
            nc.tensor.matmul(gstat[:], lhsT=ind32_t[:], rhs=stats[:],
                             start=True, stop=True)

            small = persist.tile([4, 6 * CT], F32, name="small")
            # small cols: [0:8] mu|ex2 scaled later; layout:
            #   gs   = small[:, 0:16]   (mu | ex2)
            #   mu2  = small[:, 16:24]
            #   var  = small[:, 24:32]
            #   lnv  = small[:, 32:40]
            #   rstd_nmr = small[:, 40:48] is not enough; use separate tile
            gs = small[:, 0:2 * CT]
            nc.scalar.activation(gs, gstat[:], AF.Identity, scale=1.0 / NG_ELEMS)
            mu = gs[:, 0:CT]
            ex2 = gs[:, CT:2 * CT]
            mu2 = small[:, 2 * CT:3 * CT]
            nc.vector.tensor_mul(mu2, mu, mu)
            var = small[:, 3 * CT:4 * CT]
            nc.vector.tensor_sub(var, ex2, mu2)
            lnv = small[:, 4 * CT:5 * CT]
            eps_t = persist.tile([4, 1], F32, name="eps_t")
            nc.vector.memset(eps_t[:], EPS)
            nc.scalar.activation(lnv, var, AF.Ln, bias=eps_t[:])
            rstd_nmr = persist.tile([4, 2 * CT], F32, name="rstd_nmr")
            nc.scalar.activation(rstd_nmr[:, 0:CT], lnv, AF.Exp, scale=-0.5)
            nc.vector.scalar_tensor_tensor(rstd_nmr[:, CT:2 * CT], in0=mu,
                                           scalar=-1.0,
                                           in1=rstd_nmr[:, 0:CT],
                                           op0=OP.mult, op1=OP.mult)
            abps = psA.tile([P, 2 * CT], F32, name="abps", tag="abps")
            nc.tensor.matmul(abps[:], lhsT=i2bc_t[:], rhs=rstd_nmr[:],
                             start=True, stop=True)
            scale_c = persist.tile([P, CT], F32, name="scale_c")
            nc.vector.tensor_mul(scale_c[:], abps[:, 0:CT], gnw_t[:])
            bias_c = persist.tile([P, CT], F32, name="bias_c")
            nc.vector.tensor_mul(bias_c[:], abps[:, CT:2 * CT], gnw_t[:])
            nc.vector.tensor_add(bias_c[:], bias_c[:], gnb_t[:])

            # normalize in place: h = x * scale_c + bias_c  (per channel)
            ht = xt
            for j in range(CT):
                nc.vector.tensor_scalar(_r(ht[j][:]), xt[j][:],
                                        scale_c[:, j:j + 1],
                                        bias_c[:, j:j + 1],
                                        op0=OP.mult, op1=OP.add)
            pool_junk.release()
            psA.release()

            # ---------------- phase B: qkv = Wqkv @ h + b
            psB = tc.alloc_tile_pool(name="psB", bufs=2, space="PSUM")
            qkv = [persist.tile([P, T], F32, name=f"qkv{m}", tag=f"qkv{m}")
                   for m in range(QT)]
            for m in range(QT):
                for n in range(NTC):
                    ps = psB.tile([P, TC], F32, name="qkvps", tag="qkvps")
                    for k in range(CT):
                        nc.tensor.matmul(
                            ps[:],
                            lhsT=wq[k][:, m * P:(m + 1) * P],
                            rhs=_r(ht[k][:, n * TC:(n + 1) * TC]),
                            start=(k == 0), stop=(k == CT - 1))
                    nc.scalar.activation(_r(qkv[m][:, n * TC:(n + 1) * TC]), ps[:],
                                         AF.Identity, bias=bq_t[:, m:m + 1])
            psB.release()
            pool_w.release()
            pool_x.release()

            # mask folded into v (post-softmax mask == mask on v columns)
            for vp in (2, 5):
                nc.vector.tensor_mul(_r(qkv[vp][:]), qkv[vp][:], maskb[:])

            # ---------------- phase C: attention per head
            psC = tc.alloc_tile_pool(name="psC", bufs=1, space="PSUM")
            attn = tc.alloc_tile_pool(name="attn", bufs=1)
            a_all = [persist.tile([P, T], F32, name=f"a_all{k}", tag=f"a{k}")
                     for k in range(2)]

            for l in range(HPC):             # local head
                pr, hh = divmod(l, 2)        # pair, half
                qtile, ktile, vtile = qkv[3 * pr], qkv[3 * pr + 1], qkv[3 * pr + 2]
                rs = slice(hh * CH, (hh + 1) * CH)      # partition slice
                idn = ident_t[rs, rs]

                # v^T tiles with ones column for the softmax denominator
                vta = []
                for s in range(ST):
                    tp = psC.tile([P, CH], F32, name="vtps", tag="aps", bufs=2)
                    nc.tensor.transpose(tp[:].bitcast(F32R) if USE_FP32R
                                        else tp[:],
                                        _r(vtile[rs, s * P:(s + 1) * P]),
                                        idn)
                    vt = attn.tile([P, CH + 1], F32, name="vta", tag="vta",
                                   bufs=2 * ST)
                    nc.vector.tensor_copy(_r(vt[:, 0:CH]), tp[:])
                    nc.vector.tensor_copy(_r(vt[:, CH:CH + 1]), ones_c[:])
                    vta.append(vt)

                for n in range(NTCB):        # 1024-wide t chunks
                    tsl = slice(n * TCB, (n + 1) * TCB)
                    # scoresT -> exp
                    expts = []
                    for s in range(ST):
                        sps = psC.tile([P, TCB], F32, name="sps", tag="sps",
                                       bufs=2)
                        for hf in range(2):
                            nc.tensor.matmul(
                                sps[:, hf * TC:(hf + 1) * TC],
                                lhsT=_r(ktile[rs, s * P:(s + 1) * P]),
                                rhs=_r(qtile[rs, n * TCB + hf * TC:
                                             n * TCB + (hf + 1) * TC]),
                                start=True, stop=True)
                        et = attn.tile([P, TCB], F32, name="expt", tag="expt",
                                       bufs=16)
                        nc.scalar.activation(_r(et[:]), sps[:], AF.Exp, scale=0.125)
                        expts.append(et)
                    # a~ = [v;1]^T @ exp   (accumulate over s)
                    aps = psC.tile([CH + 1, TCB], F32, name="aps", tag="aps",
                                   bufs=2)
                    for s in range(ST):
                        for hf in range(2):
                            nc.tensor.matmul(
                                aps[:, hf * TC:(hf + 1) * TC],
                                lhsT=_r(vta[s][:]),
                                rhs=_r(expts[s][:, hf * TC:(hf + 1) * TC]),
                                start=(s == 0), stop=(s == ST - 1))
                    # normalize by the denominator row
                    rec = attn.tile([1, TCB], F32, name="rec", tag="rec", bufs=2)
                    nc.vector.reciprocal(rec[:], aps[CH:CH + 1, :])
                    rb = attn.tile([CH, TCB], F32, name="rb", tag="rb", bufs=2)
                    nc.gpsimd.partition_broadcast(rb[:], rec[:])
                    if hh == 0:
                        nc.vector.tensor_mul(_r(a_all[pr][0:CH, tsl]),
                                             aps[0:CH, :], rb[:])
                    else:
                        bsh = attn.tile([CH, TCB], F32, name="bsh", tag="bsh",
                                        bufs=2)
                        nc.vector.tensor_mul(_r(bsh[:]), aps[0:CH, :], rb[:])
                        nc.sync.dma_start(_r(a_all[pr][CH:P, tsl]), _r(bsh[:]))
            attn.release()
            psC.release()

            # ---------------- phase D: partial projection
            psD = tc.alloc_tile_pool(name="psD", bufs=4, space="PSUM")
            outp = tc.alloc_tile_pool(name="outp", bufs=2)
            for m in range(CT):
                ot = outp.tile([P, T], F32, name="ot", tag="ot")
                for n in range(NTC):
                    pps = psD.tile([P, TC], F32, name="pps", tag="pps")
                    for k in range(2):
                        nc.tensor.matmul(
                            pps[:],
                            lhsT=pj[k][:, m * P:(m + 1) * P],
                            rhs=_r(a_all[k][:, n * TC:(n + 1) * TC]),
                            start=(k == 0), stop=(k == 1))
                    if n % 2 == 0:
                        nc.vector.tensor_copy(ot[:, n * TC:(n + 1) * TC], pps[:])
                    else:
                        nc.scalar.copy(ot[:, n * TC:(n + 1) * TC], pps[:])
                nc.sync.dma_start(out_d[m * P:(m + 1) * P, :], ot[:])
            outp.release()
            psD.release()

    nc.compile()
    return nc


# ---------------------------------------------------------------- host side
def _consts():
    ident = np.eye(P, dtype=np.float32)
    ind32 = np.zeros((P, 4), dtype=np.float32)
    for p in range(P):
        ind32[p, p // 32] = 1.0
    i2bc = np.ascontiguousarray(ind32.T)
    return ident, ind32, i2bc


def _perm_for(hp):
    perm = []
    for pr in range(2):
        for part in range(3):
            for hh in range(2):
                g = HPC * hp + 2 * pr + hh
                base = 192 * g + CH * part
                perm.extend(range(base, base + CH))
    return np.array(perm)


def make_in_maps(x, mask, qkv_w, qkv_b, proj_w, gn_w, gn_b):
    ident, ind32, i2bc = _consts()
    gnw_t = np.ascontiguousarray(gn_w.reshape(CT, P).T)
    gnb_t = np.ascontiguousarray(gn_b.reshape(CT, P).T)
    in_maps = []
    for i in range(NCORES):
        bb, hp = divmod(i, GPC)
        perm = _perm_for(hp)
        in_maps.append({
            "x": np.ascontiguousarray(x[bb]),
            "wqkvT": np.ascontiguousarray(qkv_w[perm, :].T),
            "bqkvT": np.ascontiguousarray(qkv_b[perm].reshape(QT, P).T),
            "projT": np.ascontiguousarray(
                proj_w[:, ASL * hp:ASL * (hp + 1)].T),
            "mask": np.ascontiguousarray(mask[bb:bb + 1]),
            "gnw": gnw_t,
            "gnb": gnb_t,
            "ident": ident,
            "ind32": ind32,
            "i2bc": i2bc,
        })
    return in_maps


_NC = None


def _get_nc():
    global _NC
    if _NC is None:
        _NC = build_program()
    return _NC


def kernel(x, mask, qkv_w, qkv_b, proj_w, proj_b, gn_w, gn_b):
    x = np.asarray(x, dtype=np.float32)
    mask = np.asarray(mask, dtype=np.float32)
    qkv_w = np.asarray(qkv_w, dtype=np.float32)
    qkv_b = np.asarray(qkv_b, dtype=np.float32)
    proj_w = np.asarray(proj_w, dtype=np.float32)
    proj_b = np.asarray(proj_b, dtype=np.float32)
    gn_w = np.asarray(gn_w, dtype=np.float32)
    gn_b = np.asarray(gn_b, dtype=np.float32)

    nc = _get_nc()
    in_maps = make_in_maps(x, mask, qkv_w, qkv_b, proj_w, gn_w, gn_b)
    res = run_bass_kernel_spmd(nc, in_maps, list(range(NCORES)))
    out = np.empty((B, C, T), dtype=np.float32)
    for bb in range(B):
        acc = x[bb] + proj_b[:, None]
        for hp in range(GPC):
            acc = acc + res.results[bb * GPC + hp]["out"]
        out[bb] = acc
    return out


# revision 31
# speedup vs baseline: 52.1180x; 52.1180x over previous
"""Trainium2 Bass kernel for nn_AttentionBlock (B=2, C=1024, T=2048, H=16, GN32).

Sharding: B*H = 32 heads across 8 cores -> 4 heads/core (core i: batch i//4,
heads 4*(i%4) .. 4*(i%4)+3).  Each core:
  - computes GroupNorm(x[b]) fully (stats replicated per batch group),
  - computes its 768 qkv rows (weights pre-transposed+permuted on host),
  - attention per head in transposed-score layout: scoresT[s,t] = k^T q,
    exp on ScalarE (scale=1/8 folded in), softmax denominator obtained by
    appending a ones-column to v^T in the V-matmul, mask folded into v,
    normalization applied post-V-matmul (deferred divide),
  - partial projection proj_w[:, slice] @ a_slice  -> [1024, 2048].
Host sums the 4 partials per batch + residual + proj bias.
"""

import math

import numpy as np

import concourse.bass as bass
import concourse.tile as tile
from concourse import bacc, mybir
from concourse.bass_utils import run_bass_kernel_spmd

# ---------------------------------------------------------------- constants
B, C, T, H = 2, 1024, 2048, 16
GROUPS = 32
EPS = 1e-5
CH = C // H              # 64 head dim
P = 128
NCORES = 8
GPC = NCORES // B        # 4 cores per batch sample
HPC = H // GPC           # 4 heads per core
CT = C // P              # 8 channel tiles
QK_ROWS = HPC * 2 * CH   # 512 q,k rows per core
QT = QK_ROWS // P        # 4 qk row tiles
WV_COLS = HPC * CH       # 256 v columns
ASL = HPC * CH           # 256 local a-channels
TC = 512                 # matmul N chunk
NTC = T // TC            # 4
TCB = 1024               # exp / score chunk (2 psum banks)
NTCB = T // TCB          # 2
ST = T // P              # 16 s-tiles
NG_ELEMS = (C // GROUPS) * T  # elements per group norm group

F32 = mybir.dt.float32
F32R = mybir.dt.float32r
AF = mybir.ActivationFunctionType
OP = mybir.AluOpType
AX = mybir.AxisListType

USE_FP32R = True         # fast fp32 matmul mode (toggle for accuracy tests)


def _r(ap):
    return ap.bitcast(F32R) if USE_FP32R else ap


WDT = F32R if USE_FP32R else F32  # dtype for matmul-feeding weight tensors


def _emit_v(nc, aps, vta_l, pend):
    s, hf, et = pend
    vw = CH + 1
    for c2 in range(2):
        c = 2 * hf + c2
        nc.tensor.matmul(
            aps[:, c * TC:(c + 1) * TC],
            lhsT=vta_l[:, s * vw:(s + 1) * vw],
            rhs=_r(et[:, c2 * TC:(c2 + 1) * TC]),
            start=(s == 0), stop=(s == ST - 1))


# ---------------------------------------------------------------- program
def build_program(debug_outputs=False):
    nc = bacc.Bacc("TRN2", target_bir_lowering=False, debug=False,
                   num_devices=NCORES)

    x_d = nc.dram_tensor("x", [C, T], F32, kind="ExternalInput").ap()
    wq_d = nc.dram_tensor("wqkT", [C, QK_ROWS], WDT, kind="ExternalInput").ap()
    wv_d = nc.dram_tensor("wvT", [C, WV_COLS], WDT, kind="ExternalInput").ap()
    vb_d = nc.dram_tensor("vbrow", [1, WV_COLS], WDT, kind="ExternalInput").ap()
    mt_d = nc.dram_tensor("maskT", [P, 2 * ST], F32, kind="ExternalInput").ap()
    bq_d = nc.dram_tensor("bqkT", [P, QT], F32, kind="ExternalInput").ap()
    pj_d = nc.dram_tensor("projT", [ASL, C], WDT, kind="ExternalInput").ap()
    gw_d = nc.dram_tensor("gnw", [P, CT], F32, kind="ExternalInput").ap()
    gb_d = nc.dram_tensor("gnb", [P, CT], F32, kind="ExternalInput").ap()
    i32_d = nc.dram_tensor("ind32", [P, 4], F32, kind="ExternalInput").ap()
    i2_d = nc.dram_tensor("i2bc", [4, P], F32, kind="ExternalInput").ap()
    out_d = nc.dram_tensor("out", [C, T], F32, kind="ExternalOutput").ap()
    if debug_outputs:
        dbg_h = nc.dram_tensor("dbg_h", [P, T], F32, kind="ExternalOutput").ap()
        dbg_q = nc.dram_tensor("dbg_q", [P, T], F32, kind="ExternalOutput").ap()
        dbg_vta = nc.dram_tensor("dbg_vta", [P, 16 * (CH + 1)], F32,
                                 kind="ExternalOutput").ap()
        dbg_a = nc.dram_tensor("dbg_a", [P, T], F32, kind="ExternalOutput").ap()

    with tile.TileContext(nc) as tc:
        from contextlib import ExitStack
        es = ExitStack()
        with es:
            persist = es.enter_context(tc.tile_pool(name="persist", bufs=1))
            pool_x = tc.alloc_tile_pool(name="xpool", bufs=1)
            pool_w = tc.alloc_tile_pool(name="wpool", bufs=1)
            pool_junk = tc.alloc_tile_pool(name="junk", bufs=1)
            psA = tc.alloc_tile_pool(name="psA", bufs=1, space="PSUM")

            # ---------------- loads
            xt = [pool_x.tile([P, T], F32, name=f"xt{j}", tag=f"xt{j}")
                  for j in range(CT)]
            for j in range(CT):
                for hx in range(2):
                    cs = slice(hx * (T // 2), (hx + 1) * (T // 2))
                    nc.sync.dma_start(_r(xt[j][:, cs]),
                                      _r(x_d[j * P:(j + 1) * P, cs]))

            wq = [pool_w.tile([P, QK_ROWS], WDT, name=f"wq{j}", tag=f"wq{j}")
                  for j in range(CT)]
            for j in range(CT):
                nc.sync.dma_start(wq[j][:], wq_d[j * P:(j + 1) * P, :])
            wv = [pool_w.tile([P, WV_COLS], WDT, name=f"wv{j}", tag=f"wv{j}")
                  for j in range(CT)]
            for j in range(CT):
                nc.sync.dma_start(wv[j][:], wv_d[j * P:(j + 1) * P, :])
            vbrow_t = persist.tile([1, WV_COLS], WDT, name="vbrow_t")
            nc.sync.dma_start(vbrow_t[:], vb_d[:])
            ones_raw = persist.tile([1, P], F32, name="ones_raw")
            nc.vector.memset(ones_raw[:], 1.0)
            ones_r = persist.tile([1, P], WDT, name="ones_r")
            nc.vector.tensor_copy(ones_r[:], ones_raw[:])
            maskT_t = persist.tile([P, 2 * ST], F32, name="maskT_t")
            nc.sync.dma_start(maskT_t[:], mt_d[:])

            pj = [persist.tile([P, C], WDT, name=f"pj{k}", tag=f"pj{k}")
                  for k in range(2)]
            for k in range(2):
                nc.sync.dma_start(pj[k][:], pj_d[k * P:(k + 1) * P, :])

            bq_t = persist.tile([P, QT], F32, name="bq_t")
            nc.sync.dma_start(bq_t[:], bq_d[:])
            gnw_t = persist.tile([P, CT], F32, name="gnw_t")
            nc.sync.dma_start(gnw_t[:], gw_d[:])
            gnb_t = persist.tile([P, CT], F32, name="gnb_t")
            nc.sync.dma_start(gnb_t[:], gb_d[:])
            ind32_t = persist.tile([P, 4], F32, name="ind32_t")
            nc.sync.dma_start(ind32_t[:], i32_d[:])
            i2bc_t = persist.tile([4, P], F32, name="i2bc_t")
            nc.sync.dma_start(i2bc_t[:], i2_d[:])
            ones_c = persist.tile([P, 1], F32, name="ones_c")
            nc.vector.memset(ones_c[:], 1.0)

            # ---------------- phase A: group norm stats (half tiles for
            # finer DMA/compute overlap)
            NH = 2 * CT
            stats = persist.tile([P, 2 * NH], F32, name="stats")
            for j in range(CT):
                for hx in range(2):
                    i = 2 * j + hx
                    xsl = xt[j][:, hx * (T // 2):(hx + 1) * (T // 2)]
                    nc.vector.tensor_reduce(stats[:, i:i + 1], xsl,
                                            axis=AX.X, op=OP.add)
                    junk = pool_junk.tile([P, T // 2], F32, name="junk",
                                          tag="junk")
                    nc.scalar.activation(junk[:], xsl, AF.Square,
                                         accum_out=stats[:, NH + i:NH + i + 1])

            gstat = psA.tile([4, 2 * NH], F32, name="gstat", tag="gstat")
            nc.tensor.matmul(gstat[:], lhsT=ind32_t[:], rhs=stats[:],
                             start=True, stop=True)
            # scale to means and move to SBUF (DVE may read only one PSUM
            # operand), then combine half-tile sums
            gs32 = persist.tile([4, 2 * NH], F32, name="gs32")
            nc.scalar.activation(gs32[:], gstat[:], AF.Identity,
                                 scale=1.0 / NG_ELEMS)

            small = persist.tile([4, 6 * CT], F32, name="small")
            # small cols: [0:8] mu|ex2 scaled later; layout:
            #   gs   = small[:, 0:16]   (mu | ex2)
            #   mu2  = small[:, 16:24]
            #   var  = small[:, 24:32]
            #   lnv  = small[:, 32:40]
            #   rstd_nmr = small[:, 40:48] is not enough; use separate tile
            gs = small[:, 0:2 * CT]
            nc.vector.tensor_add(
                gs,
                gs32[:].rearrange("p (i two) -> p i two", two=2)[:, :, 0],
                gs32[:].rearrange("p (i two) -> p i two", two=2)[:, :, 1])
            mu = gs[:, 0:CT]
            ex2 = gs[:, CT:2 * CT]
            mu2 = small[:, 2 * CT:3 * CT]
            nc.vector.tensor_mul(mu2, mu, mu)
            var = small[:, 3 * CT:4 * CT]
            nc.vector.tensor_sub(var, ex2, mu2)
            lnv = small[:, 4 * CT:5 * CT]
            eps_t = persist.tile([4, 1], F32, name="eps_t")
            nc.vector.memset(eps_t[:], EPS)
            nc.scalar.activation(lnv, var, AF.Ln, bias=eps_t[:])
            rstd_nmr = persist.tile([4, 2 * CT], F32, name="rstd_nmr")
            nc.scalar.activation(rstd_nmr[:, 0:CT], lnv, AF.Exp, scale=-0.5)
            nc.vector.scalar_tensor_tensor(rstd_nmr[:, CT:2 * CT], in0=mu,
                                           scalar=-1.0,
                                           in1=rstd_nmr[:, 0:CT],
                                           op0=OP.mult, op1=OP.mult)
            abps = psA.tile([P, 2 * CT], F32, name="abps", tag="abps")
            nc.tensor.matmul(abps[:], lhsT=i2bc_t[:], rhs=rstd_nmr[:],
                             start=True, stop=True)
            scale_c = persist.tile([P, CT], F32, name="scale_c")
            nc.vector.tensor_mul(scale_c[:], abps[:, 0:CT], gnw_t[:])
            bias_c = persist.tile([P, CT], F32, name="bias_c")
            nc.vector.tensor_mul(bias_c[:], abps[:, CT:2 * CT], gnw_t[:])
            nc.vector.tensor_add(bias_c[:], bias_c[:], gnb_t[:])

            # normalize in place: h = x * scale_c + bias_c  (per channel)
            ht = xt
            for j in range(CT):
                nc.vector.tensor_scalar(_r(ht[j][:]), xt[j][:],
                                        scale_c[:, j:j + 1],
                                        bias_c[:, j:j + 1],
                                        op0=OP.mult, op1=OP.add)
            pool_junk.release()
            psA.release()

            # ---------------- phase B: qkv = Wqkv @ h + b
            psB = tc.alloc_tile_pool(name="psB", bufs=2, space="PSUM")
            qkv = [persist.tile([P, T], F32, name=f"qkv{m}", tag=f"qkv{m}")
                   for m in range(QT)]
            for m in range(QT):
                for n in range(NTC):
                    ps = psB.tile([P, TC], F32, name="qkvps", tag="qkvps")
                    for k in range(CT):
                        nc.tensor.matmul(
                            ps[:],
                            lhsT=wq[k][:, m * P:(m + 1) * P],
                            rhs=_r(ht[k][:, n * TC:(n + 1) * TC]),
                            start=(k == 0), stop=(k == CT - 1))
                    nc.vector.tensor_scalar(
                        _r(qkv[m][:, n * TC:(n + 1) * TC]), ps[:],
                        bq_t[:, m:m + 1], None, op0=OP.add)
            # ---------------- phase B2: vT tiles directly from h
            # vta[l][s] columns: [0:64] v*mask (transposed), 64: ones -> D,
            # 65: mask -> Dm.  v bias folded in later: a~ + b_v * Dm.
            VW = CH + 1
            attn_v = tc.alloc_tile_pool(name="attn_v", bufs=1, side="right")
            vta = [attn_v.tile([P, ST * VW], WDT, name=f"vta{l}",
                               tag=f"vta{l}") for l in range(HPC)]
            for s in range(ST):
                vtp = psB.tile([P, WV_COLS], F32, name="vtp", tag="vtp", bufs=2)
                for k in range(CT):
                    nc.tensor.matmul(
                        vtp[:],
                        lhsT=_r(ht[k][:, s * P:(s + 1) * P]),
                        rhs=wv[k][:],
                        start=(k == 0), stop=False)
                nc.tensor.matmul(
                    vtp[:], lhsT=ones_r[:], rhs=vbrow_t[:],
                    start=False, stop=True)
                for l in range(HPC):
                    hh = l % 2
                    vt = vta[l][:, s * VW:(s + 1) * VW]
                    # legacy tile() quirk: head g uses mask[g % B]
                    ms = hh * ST + s
                    nc.vector.tensor_scalar(
                        _r(vt[:, 0:CH]), vtp[:, l * CH:(l + 1) * CH],
                        maskT_t[:, ms:ms + 1], None, op0=OP.mult)
                    nc.vector.tensor_copy(_r(vt[:, CH:CH + 1]), ones_c[:])
            psB.release()
            pool_w.release()
            pool_x.release()

            # ---------------- phase C: attention per head
            # psD first: its pps tiles must not wait for psC's release, so
            # the pair-0 projection pass can overlap heads 2-3.
            psD = tc.alloc_tile_pool(name="psD", bufs=1, space="PSUM")
            psC = tc.alloc_tile_pool(name="psC", bufs=1, space="PSUM")
            outp = tc.alloc_tile_pool(name="outp", bufs=1)
            attn = tc.alloc_tile_pool(name="attn", bufs=1)
            a_all = [persist.tile([P, T], F32, name=f"a_all{k}", tag=f"a{k}")
                     for k in range(2)]

            for l in range(HPC):             # local head
                pr, hh = divmod(l, 2)        # pair, half
                qtile, ktile = qkv[2 * pr], qkv[2 * pr + 1]
                rs = slice(hh * CH, (hh + 1) * CH)      # partition slice

                for hf in range(NTCB):
                    apq = [psC.tile([CH + 1, TC], F32, name=f"apq{c2}",
                                    tag="aps", bufs=3) for c2 in range(2)]
                    for s in range(ST):
                        sps = psC.tile([P, TCB], F32, name="sps", tag="sps",
                                       bufs=2)
                        for c2 in range(2):
                            c = 2 * hf + c2
                            nc.tensor.matmul(
                                sps[:, c2 * TC:(c2 + 1) * TC],
                                lhsT=_r(ktile[rs, s * P:(s + 1) * P]),
                                rhs=_r(qtile[rs, c * TC:(c + 1) * TC]),
                                start=True, stop=True)
                        et = attn.tile([P, TCB], F32, name="expt", tag="expt",
                                       bufs=6)
                        nc.scalar.activation(_r(et[:]), sps[:], AF.Exp,
                                             scale=0.125)
                        for c2 in range(2):
                            nc.tensor.matmul(
                                apq[c2][:],
                                lhsT=vta[l][:, s * VW:(s + 1) * VW],
                                rhs=_r(et[:, c2 * TC:(c2 + 1) * TC]),
                                start=(s == 0), stop=(s == ST - 1))
                    for c2 in range(2):
                        aps = apq[c2]
                        c = 2 * hf + c2
                        tsl = slice(c * TC, (c + 1) * TC)
                        rec = attn.tile([1, TC], F32, name="rec", tag="rec",
                                        bufs=3)
                        nc.vector.reciprocal(rec[:], aps[CH:CH + 1, :])
                        rb = attn.tile([CH, TC], F32, name="rb", tag="rb",
                                       bufs=3)
                        nc.gpsimd.partition_broadcast(rb[:], rec[:])
                        if hh == 0:
                            nc.vector.tensor_mul(_r(a_all[pr][0:CH, tsl]),
                                                 aps[0:CH, :], rb[:])
                        else:
                            bsh = attn.tile([CH, TC], F32, name="bsh",
                                            tag="bsh", bufs=3)
                            nc.vector.tensor_mul(_r(bsh[:]), aps[0:CH, :],
                                                 rb[:])
                            nc.sync.dma_start(_r(a_all[pr][CH:P, tsl]),
                                              _r(bsh[:]))
            attn.release()
            attn_v.release()
            psC.release()

            if debug_outputs:
                nc.sync.dma_start(_r(dbg_h[:]), _r(ht[0][:]))
                nc.sync.dma_start(_r(dbg_q[:]), _r(qkv[0][:]))
                nc.sync.dma_start(dbg_vta[:].bitcast(WDT), vta[0][:])
                nc.sync.dma_start(_r(dbg_a[:]), _r(a_all[0][:]))

            # ---------------- phase D: partial projection
            ots = [outp.tile([P, T], F32, name=f"ot{m}", tag=f"ot{m}")
                   for m in range(CT)]
            for m in range(CT):
                for n in range(NTC):
                    pps = psD.tile([P, TC], F32, name="pps", tag="pps")
                    nc.tensor.matmul(pps[:],
                                     lhsT=pj[0][:, m * P:(m + 1) * P],
                                     rhs=_r(a_all[0][:, n * TC:(n + 1) * TC]),
                                     start=True, stop=True)
                    if n % 2 == 0:
                        nc.vector.tensor_copy(ots[m][:, n * TC:(n + 1) * TC],
                                              pps[:])
                    else:
                        nc.scalar.copy(ots[m][:, n * TC:(n + 1) * TC], pps[:])
            psD2 = tc.alloc_tile_pool(name="psD2", bufs=4, space="PSUM")
            for m in range(CT):
                for n in range(NTC):
                    pps = psD2.tile([P, TC], F32, name="pps2", tag="pps2")
                    nc.tensor.matmul(pps[:],
                                     lhsT=pj[1][:, m * P:(m + 1) * P],
                                     rhs=_r(a_all[1][:, n * TC:(n + 1) * TC]),
                                     start=True, stop=True)
                    nc.vector.tensor_add(ots[m][:, n * TC:(n + 1) * TC],
                                         ots[m][:, n * TC:(n + 1) * TC],
                                         pps[:])
                    if n % 2 == 1:
                        cs = slice((n - 1) * TC, (n + 1) * TC)
                        nc.sync.dma_start(out_d[m * P:(m + 1) * P, cs],
                                          ots[m][:, cs])
            outp.release()
            psD2.release()
            psD.release()

    nc.compile()
    return nc


# ---------------------------------------------------------------- host side
def _consts():
    ind32 = np.zeros((P, 4), dtype=np.float32)
    for p in range(P):
        ind32[p, p // 32] = 1.0
    i2bc = np.ascontiguousarray(ind32.T)
    return ind32, i2bc


def _perm_qk(hp):
    perm = []
    for pr in range(2):
        for part in range(2):          # q then k
            for hh in range(2):
                g = HPC * hp + 2 * pr + hh
                base = 192 * g + CH * part
                perm.extend(range(base, base + CH))
    return np.array(perm)


def _perm_v(hp):
    perm = []
    for l in range(HPC):
        g = HPC * hp + l
        perm.extend(range(192 * g + 2 * CH, 192 * g + 3 * CH))
    return np.array(perm)


def make_in_maps(x, mask, qkv_w, qkv_b, proj_w, gn_w, gn_b):
    ind32, i2bc = _consts()
    gnw_t = np.ascontiguousarray(gn_w.reshape(CT, P).T)
    gnb_t = np.ascontiguousarray(gn_b.reshape(CT, P).T)
    in_maps = []
    for i in range(NCORES):
        bb, hp = divmod(i, GPC)
        pq = _perm_qk(hp)
        pv = _perm_v(hp)
        in_maps.append({
            "x": np.ascontiguousarray(x[bb]),
            "wqkT": np.ascontiguousarray(qkv_w[pq, :].T),
            "bqkT": np.ascontiguousarray(qkv_b[pq].reshape(QT, P).T),
            "wvT": np.ascontiguousarray(qkv_w[pv, :].T),
            "vbrow": np.ascontiguousarray(qkv_b[pv][None, :]),
            "projT": np.ascontiguousarray(
                proj_w[:, ASL * hp:ASL * (hp + 1)].T),
            "maskT": np.ascontiguousarray(
                np.concatenate([mask[0].reshape(ST, P).T,
                                mask[1].reshape(ST, P).T], axis=1)),
            "gnw": gnw_t,
            "gnb": gnb_t,
            "ind32": ind32,
            "i2bc": i2bc,
        })
    return in_maps


_NC = None


def _get_nc():
    global _NC
    if _NC is None:
        _NC = build_program()
    return _NC


def kernel(x, mask, qkv_w, qkv_b, proj_w, proj_b, gn_w, gn_b):
    x = np.asarray(x, dtype=np.float32)
    mask = np.asarray(mask, dtype=np.float32)
    qkv_w = np.asarray(qkv_w, dtype=np.float32)
    qkv_b = np.asarray(qkv_b, dtype=np.float32)
    proj_w = np.asarray(proj_w, dtype=np.float32)
    proj_b = np.asarray(proj_b, dtype=np.float32)
    gn_w = np.asarray(gn_w, dtype=np.float32)
    gn_b = np.asarray(gn_b, dtype=np.float32)

    nc = _get_nc()
    in_maps = make_in_maps(x, mask, qkv_w, qkv_b, proj_w, gn_w, gn_b)
    res = run_bass_kernel_spmd(nc, in_maps, list(range(NCORES)))
    out = np.empty((B, C, T), dtype=np.float32)
    for bb in range(B):
        acc = x[bb] + proj_b[:, None]
        for hp in range(GPC):
            acc = acc + res.results[bb * GPC + hp]["out"]
        out[bb] = acc
    return out
